# revision 18
# baseline (speedup 1.0000x reference)
"""BiSRU Trainium2 kernel.

Reference computation (T=2048, B=16, D=1024):
    pre = einsum('tbi,io->tbo', x, W)                  # [T,B,3D]
    pre = LayerNorm(pre) * gamma + beta                # over last dim
    g  = sigmoid(pre[..., :D]); xm = pre[..., D:2D]; hg = sigmoid(pre[..., 2D:])
    h_f = linrec(1-gf, gf*xf)  (forward over t, first D/2 channels)
    h_b = linrec(1-gb, gb*xb)  (backward over t, last D/2 channels)
    out = (1-hg)*[h_f, h_b] + x*hg

Sharding: batch (dim 1) across 8 cores, 2 batch elements per core, no
cross-core communication. Host pre-transposes x to [b, D, T] fp16 per core so
the matmul's contraction dim (D) lands on SBUF partitions with no on-chip
transposes (fp16 operands run the PE at full rate, 1 cycle/row). The matmul
inner loop is ko-outer over nch-triples with 3 PSUM banks accumulating in
parallel, so the stationary operand (the activation tile) is reused across 3
consecutive matmuls and LDWEIGHTS amortizes 3x. LayerNorm stats come from
bn_stats/bn_aggr; the LN sqrt is batched per time quarter ([128,4] strip) so
the ACT engine switches activation-function tables twice per quarter instead
of twice per 128-token tile (each table switch costs ~1.3us); the LN+sigmoid
gate evaluation is fused into ACT activations via per-partition scale/bias.
Scan-side arrays (g, xn, hg) take one DRAM round trip in fp16 and come back
through the DMA transpose engine in [channel, time] layout, where
tensor_tensor_scan runs the recurrence along the free (time) axis in fp32
state; the backward direction uses negative-stride APs. The gate g (not
a=1-g) is stored so the a~1 long-memory regime keeps relative precision; a
is rebuilt in fp32 on chip. Phase 2 is emitted per time-quarter interleaved
with phase 1 so it streams right behind production; backward-direction
inputs are prefetched and its chain runs in reverse quarter order at the
tail. Output is written fp16 (halves output DMA) and upconverted on host.
"""

import os

import numpy as np
import ml_dtypes

import concourse.bass as bass
import concourse.mybir as mybir
from concourse import bacc
import concourse.tile as tile
from concourse.alu_op_type import AluOpType
from concourse.bass_utils import run_bass_kernel_spmd

F32 = mybir.dt.float32
F32R = mybir.dt.float32r
F16 = mybir.dt.float16
F16_NP = np.float16

T, B, D = 2048, 16, 1024
ND = 3 * D
NCORES = 8
BL = B // NCORES  # batch per core
EPS = 1e-5
P = 128
NCH = ND // 512       # 6 matmul output chunks of 512
KO = D // P           # 8 contraction subtiles
TT = T // P           # 16 token tiles per batch element
HALF = D // 2

LAST_RESULTS = None  # BassKernelResults of the most recent run (for test.py)

_PROG_CACHE = {}


def _build_program(general_ln: bool, reps: int = 1, phases=(1, 2)) -> bass.Bass:
    nc = bacc.Bacc()

    xT = nc.declare_dram_parameter("xT", [BL, D, T], F16, isOutput=False)
    W = nc.declare_dram_parameter("W", [D, ND], F16, isOutput=False)
    if general_ln:
        gamma = nc.declare_dram_parameter("gamma", [ND], F32, isOutput=False)
        beta = nc.declare_dram_parameter("beta", [ND], F32, isOutput=False)
    outT = nc.declare_dram_parameter("outT", [BL, D, T], F16, isOutput=True)

    with tile.TileContext(nc) as tc:
        with (
            tc.tile_pool(name="singles", bufs=1) as singles,
            tc.tile_pool(name="dram", bufs=1, space="DRAM") as dram,
            tc.tile_pool(name="lx", bufs=5) as lxp,
            tc.tile_pool(name="pre", bufs=5) as prep,
            tc.tile_pool(name="stats", bufs=4) as statp,
            tc.tile_pool(name="gates", bufs=2) as gatep,
            tc.tile_pool(name="p2", bufs=4) as p2p,
            tc.tile_pool(name="p2h", bufs=8) as p2hp,
            tc.tile_pool(name="bw", bufs=4) as bwp,
            tc.tile_pool(name="out", bufs=3) as outp,
            tc.tile_pool(name="psum", bufs=8, space="PSUM") as psum,
        ):
            # ---- constants / weights resident in SBUF ----
            W_sb = singles.tile([P, KO, ND], F16)
            W_r = W.rearrange("(ko p) n -> p ko n", p=P)
            W_loaded = [False]

            def load_W():
                if not W_loaded[0]:
                    W_loaded[0] = True
                    for nch in range(NCH):
                        nc.sync.dma_start(
                            W_sb[:, :, nch * 512 : (nch + 1) * 512],
                            W_r[:, :, nch * 512 : (nch + 1) * 512],
                        )
            eps_sb = singles.tile([P, 1], F32)
            nc.vector.memset(eps_sb, EPS)
            if general_ln:
                # gamma/beta broadcast to all 128 partitions
                gam_sb = singles.tile([P, ND], F16)
                bet_sb = singles.tile([P, ND], F16)
                nc.sync.dma_start(gam_sb, gamma.to_broadcast((P, ND)))
                nc.sync.dma_start(bet_sb, beta.to_broadcast((P, ND)))

            # ---- DRAM scratch (fp16), per batch element and time-quarter ----
            NQ = 4                  # quarters of the time axis
            QT = T // NQ            # 512 timesteps per quarter
            a_scr = [
                [dram.tile([QT, D], F16, tag=f"a{b}q{q}", name=f"a_scr{b}q{q}")
                 for q in range(NQ)]
                for b in range(BL)
            ]
            xn_scr = [
                [dram.tile([QT, D], F16, tag=f"x{b}q{q}", name=f"xn_scr{b}q{q}")
                 for q in range(NQ)]
                for b in range(BL)
            ]
            hg_scr = [
                [dram.tile([QT, D], F16, tag=f"h{b}q{q}", name=f"hg_scr{b}q{q}")
                 for q in range(NQ)]
                for b in range(BL)
            ]

            for _rep in range(reps):
              xq_all = {}
              if 1 in phases:
                  for bb in range(BL):
                      xTr_b = xT[bb].rearrange("(ko p) t -> p ko t", p=P)
                      for q in (0, 3, 1, 2):
                          xq = lxp.tile([P, KO, T // 4], F16, tag="xq",
                                        name=f"xq_{_rep}_{bb}_{q}")
                          for hh in range(2):
                              nc.sync.dma_start(
                                  xq[:, :, hh * (T // 8) : (hh + 1) * (T // 8)],
                                  xTr_b[
                                      :,
                                      :,
                                      q * (T // 4) + hh * (T // 8) : q * (T // 4)
                                      + (hh + 1) * (T // 8),
                                  ],
                              )
                          xq_all[(bb, q)] = xq
                          if bb == 0 and q == 0:
                              load_W()
              for b in range(BL):
                  QTT = TT // 4  # token tiles per quarter
                  xq_tiles = {q: xq_all[(b, q)] for q in range(4)}

                  def p1_tile(tt, mvq, qi):
                      q4, toff = divmod(tt * P, T // 4)
                      lx = xq_tiles[q4][:, :, toff : toff + P]
                      pre_sb = prep.tile([P, NCH, 512], F16, tag="pre")
                      for g2 in range(2):
                          ps = [
                              psum.tile([P, 512], F32, tag="ps", name=f"ps{j}")
                              for j in range(3)
                          ]
                          for ko in range(KO):
                              for j in range(3):
                                  nc.tensor.matmul(
                                      ps[j],
                                      lhsT=lx[:, ko, :],
                                      rhs=W_sb[
                                          :,
                                          ko,
                                          (3 * g2 + j) * 512 : (3 * g2 + j + 1)
                                          * 512,
                                      ],
                                      start=(ko == 0),
                                      stop=(ko == KO - 1),
                                  )
                          for j in range(3):
                              nc.scalar.copy(pre_sb[:, 3 * g2 + j, :], ps[j])

                      st = statp.tile([P, NCH, 6], F32, tag="bst")
                      for nch in range(NCH):
                          nc.vector.bn_stats(st[:, nch, :], pre_sb[:, nch, :])
                      nc.vector.bn_aggr(mvq[:, qi, :], st)
                      return pre_sb

                  def gates_tile(tt, pre_sb, mvq, qi, rsq):
                      mean = mvq[:, qi, 0:1]
                      rs = rsq[:, qi : qi + 1]
                      a_t = gatep.tile([P, D], F16, tag="a")
                      xn_t = gatep.tile([P, D], F16, tag="xn")
                      hg_t = gatep.tile([P, D], F16, tag="hg")
                      if not general_ln:
                          pb = statp.tile([P, 1], F32, tag="pb")
                          nc.vector.tensor_tensor(pb, mean, rs, AluOpType.mult)
                          nb = statp.tile([P, 1], F32, tag="nb")
                          nc.vector.tensor_scalar_mul(nb, pb, -1.0)
                          for i in range(2):
                              sl = slice(i * 512, (i + 1) * 512)
                              # g = sigmoid((z-mu)*rs); stored (not a=1-g) so
                              # the a~1 regime keeps relative precision in fp16
                              nc.scalar.activation(
                                  a_t[:, sl],
                                  pre_sb[:, i, :],
                                  mybir.ActivationFunctionType.Sigmoid,
                                  bias=nb,
                                  scale=rs,
                              )
                              nc.scalar.activation(
                                  hg_t[:, sl],
                                  pre_sb[:, 4 + i, :],
                                  mybir.ActivationFunctionType.Sigmoid,
                                  bias=nb,
                                  scale=rs,
                              )
                              nc.vector.tensor_scalar(
                                  xn_t[:, sl],
                                  pre_sb[:, 2 + i, :],
                                  scalar1=mean,
                                  scalar2=rs,
                                  op0=AluOpType.subtract,
                                  op1=AluOpType.mult,
                              )
                      else:
                          zn = gatep.tile([P, NCH, 512], F16, tag="zn")
                          for nch in range(NCH):
                              nc.vector.tensor_scalar(
                                  zn[:, nch, :],
                                  pre_sb[:, nch, :],
                                  scalar1=mean,
                                  scalar2=rs,
                                  op0=AluOpType.subtract,
                                  op1=AluOpType.mult,
                              )
                          zn2 = zn.rearrange("p a b -> p (a b)")
                          nc.vector.tensor_tensor(zn2, zn2, gam_sb, AluOpType.mult)
                          nc.vector.tensor_tensor(zn2, zn2, bet_sb, AluOpType.add)
                          nc.scalar.activation(
                              a_t,
                              zn2[:, 0:D],
                              mybir.ActivationFunctionType.Sigmoid,
                          )
                          nc.scalar.activation(
                              hg_t,
                              zn2[:, 2 * D : 3 * D],
                              mybir.ActivationFunctionType.Sigmoid,
                          )
                          nc.vector.tensor_copy(xn_t, zn2[:, D : 2 * D])

                      q, qi = divmod(tt, TT // NQ)
                      rows = slice(qi * P, (qi + 1) * P)
                      nc.sync.dma_start(a_scr[b][q][rows, :], a_t)
                      nc.sync.dma_start(xn_scr[b][q][rows, :], xn_t)
                      nc.sync.dma_start(hg_scr[b][q][rows, :], hg_t)

                  # per-(dirb, cc, q) h tiles; chained via initial
                  h_tiles = {}

                  def p2_quarter(dirb, cc, q, gT, xnT, hgP=None):
                      ch = slice(dirb * HALF + cc * P, dirb * HALF + (cc + 1) * P)
                      qsl = slice(q * QT, (q + 1) * QT)
                      # a = 1-g in fp32 (decay needs full precision)
                      a32 = p2p.tile([P, QT], F32, tag="a32")
                      nc.gpsimd.tensor_scalar(
                          a32,
                          gT,
                          scalar1=-1.0,
                          scalar2=1.0,
                          op0=AluOpType.mult,
                          op1=AluOpType.add,
                      )
                      # bneg = -g*xn, overwrites xnT in place
                      bneg = xnT
                      nc.vector.scalar_tensor_tensor(
                          bneg,
                          in0=gT,
                          scalar=-1.0,
                          in1=xnT,
                          op0=AluOpType.mult,
                          op1=AluOpType.mult,
                      )
                      # h_t = a*h_{t-1} + g*xn == (a ⊗ state) - bneg
                      hq = p2hp.tile([P, QT], F16, tag="h")
                      h_tiles[(dirb, cc, q)] = hq
                      if dirb == 0:
                          init = (
                              0.0
                              if q == 0
                              else h_tiles[(0, cc, q - 1)][:, QT - 1 : QT]
                          )
                          nc.vector.tensor_tensor_scan(
                              hq,
                              data0=a32,
                              data1=bneg,
                              initial=init,
                              op0=AluOpType.mult,
                              op1=AluOpType.subtract,
                          )
                      else:
                          init = (
                              0.0
                              if q == NQ - 1
                              else h_tiles[(1, cc, q + 1)][:, 0:1]
                          )
                          nc.vector.tensor_tensor_scan(
                              hq[:, ::-1],
                              data0=a32[:, ::-1],
                              data1=bneg[:, ::-1],
                              initial=init,
                              op0=AluOpType.mult,
                              op1=AluOpType.subtract,
                          )
                      # combine: out = hg*x + (1-hg)*h = h + hg*(x-h)
                      if hgP is None:
                          hgT = p2p.tile([P, QT], F16, tag="hgT")
                          nc.sync.dma_start_transpose(hgT, hg_scr[b][q][:, ch])
                      else:
                          hgT = hgP
                      # x in [channel, time] layout is already resident: the
                      # xq tiles' partition axis IS the D axis
                      xc = xq_tiles[q][:, (dirb * HALF + cc * P) // P, :]
                      s = p2p.tile([P, QT], F16, tag="s")
                      # in the backward tail Pool saturates; DVE has slack
                      eng_s = nc.gpsimd
                      eng_o = nc.gpsimd if dirb == 0 else nc.vector
                      eng_s.tensor_tensor(s, xc, hq, AluOpType.subtract)
                      m = s
                      nc.gpsimd.tensor_tensor(m, hgT, s, AluOpType.mult)
                      o = outp.tile([P, QT], F16, tag="o")
                      eng_o.tensor_tensor(o, m, hq, AluOpType.add)
                      nc.sync.dma_start(outT[b, ch, qsl], o)

                  bwd_pre = {}
                  for q in range(NQ if 1 in phases else 0):
                      mvq = statp.tile([P, QTT, 2], F32, tag="mvq",
                                       name=f"mvq_{b}_{q}")
                      pres = [
                          p1_tile(tt, mvq, tt - q * QTT)
                          for tt in range(q * QTT, (q + 1) * QTT)
                      ]
                      sdq = statp.tile([P, QTT], F32, tag="sdq")
                      nc.scalar.activation(
                          sdq, mvq[:, :, 1],
                          mybir.ActivationFunctionType.Sqrt, bias=eps_sb,
                      )
                      rsq = statp.tile([P, QTT], F32, tag="rsq")
                      nc.vector.reciprocal(rsq, sdq)
                      for qi in range(QTT):
                          gates_tile(q * QTT + qi, pres[qi], mvq, qi, rsq)
                      if 2 not in phases:
                          continue
                      # forward chunks stream right behind production
                      for cc in range(HALF // P):
                          ch = slice(cc * P, (cc + 1) * P)
                          gT = p2p.tile([P, QT], F16, tag="gT")
                          nc.sync.dma_start_transpose(gT, a_scr[b][q][:, ch])
                          xnT = p2p.tile([P, QT], F16, tag="xnT")
                          nc.sync.dma_start_transpose(xnT, xn_scr[b][q][:, ch])
                          p2_quarter(0, cc, q, gT, xnT)
                      # backward chunks: prefetch now, compute later in
                      # reverse-quarter order
                      for cc in range(HALF // P):
                          ch = slice(HALF + cc * P, HALF + (cc + 1) * P)
                          gT = bwp.tile([P, QT], F16, tag=f"bwg{q}",
                                        name=f"bwg_{b}_{q}_{cc}")
                          nc.sync.dma_start_transpose(gT, a_scr[b][q][:, ch])
                          xnT = bwp.tile([P, QT], F16, tag=f"bwx{q}",
                                         name=f"bwx_{b}_{q}_{cc}")
                          nc.sync.dma_start_transpose(xnT, xn_scr[b][q][:, ch])
                          if q >= NQ - 2:
                              # only the first two tail quarters benefit from
                              # hg prefetch; later ones overlap earlier compute
                              hgP = bwp.tile([P, QT], F16, tag=f"bwh{q}",
                                             name=f"bwh_{b}_{q}_{cc}")
                              nc.sync.dma_start_transpose(
                                  hgP, hg_scr[b][q][:, ch]
                              )
                          else:
                              hgP = None
                          bwd_pre[(cc, q)] = (gT, xnT, hgP)
                  if 2 in phases:
                      for q in range(NQ - 1, -1, -1):
                          for cc in range(HALF // P):
                              gT, xnT, hgP = bwd_pre[(cc, q)]
                              p2_quarter(1, cc, q, gT, xnT, hgP)
    nc.compile()
    return nc


def kernel(input, W, gamma, beta):
    global LAST_RESULTS
    input = np.ascontiguousarray(np.asarray(input, dtype=np.float32))
    W = np.ascontiguousarray(np.asarray(W, dtype=np.float32))
    gamma = np.asarray(gamma, dtype=np.float32)
    beta = np.asarray(beta, dtype=np.float32)
    assert input.shape == (T, B, D) and W.shape == (D, ND)

    general_ln = not (np.all(gamma == 1.0) and np.all(beta == 0.0))
    key = general_ln
    if key not in _PROG_CACHE:
        _PROG_CACHE[key] = _build_program(general_ln)
    nc = _PROG_CACHE[key]

    in_maps = []
    for c in range(NCORES):
        xs = input[:, c * BL : (c + 1) * BL, :]  # [T, BL, D]
        xT = np.ascontiguousarray(xs.transpose(1, 2, 0))  # [BL, D, T]
        m = {
            "xT": xT.astype(F16_NP),
            "W": W.astype(F16_NP),
        }
        if general_ln:
            m["gamma"] = gamma
            m["beta"] = beta
        in_maps.append(m)

    trace = bool(int(os.environ.get("BISRU_TRACE", "0")))
    res = run_bass_kernel_spmd(nc, in_maps, list(range(NCORES)), trace=trace)
    LAST_RESULTS = res

    out = np.empty((T, B, D), dtype=np.float32)
    for c in range(NCORES):
        oT = np.asarray(res.results[c]["outT"])  # [BL, D, T] fp16
        out[:, c * BL : (c + 1) * BL, :] = (
            oT.transpose(2, 0, 1).astype(np.float32)
        )
    return out

